# revision 77
# baseline (speedup 1.0000x reference)
"""Multi-head causal linear attention (B=1, N=2048, D=1024, H=16) on 8 trn2 cores.

Math: reference computes, per head (e=64):
    q = softmax(q_raw, -1) * e**-0.5 ;  k = exp(k_raw)
    out_n = (q_n . KV_n) / (q_n . (kcum_n + EPS)),  KV_n = sum_{j<=n} k_j v_j^T
Because both numerator and denominator are linear in q_n, the softmax
normalization and the e**-0.5 scale cancel exactly; only u = exp(q_raw)
matters.  The EPS term contributes <1e-6 relative and is dropped.  The
v-bias contribution factors out:  out += b_v  (sum_j s_nj / denom ~= 1).

Per-core work (head-parallel, 2 heads/core):
    qvk^T = W_c^T @ x  computed as matmul(lhsT=W_block, rhs=x^T) on PE,
    x^T is pre-transposed on the host so no on-chip transpose is needed.
    Chunked causal linear attention (chunk=128) with the classic
    intra (masked QK^T V) + inter (running KV state) recurrence.

Layout tricks:
  - input DMA is issued first: cf on the scalar HWDGE ring, the heavy
    stream on sync in consumption order as [k0 bundle] + k-pairs + xt
    tiles.  Each dma_start costs ~600ns of serial DIRECT2D descriptor-gen
    on its ring, and consumers gate on the transfer's completion sem
    (last byte + ~1-2us receipt), so the first transfer is small and the
    rest are paired.
  - N=512 junk matmuls bridge queue-start (~7.5us of runtime boot) to the
    k0 sem so the HAM clock is warm when real work begins; the ACT exp
    table is pre-loaded off the critical path.
  - the first token tile's projection runs k-outer / f-inner so the PE
    starts as soon as bundle 0 lands; ALL of tile-0's chain is emitted
    before any tile-1 projection matmul (the Tensor queue is strict FIFO
    — an LDWEIGHTS waiting on xt1's sem at the queue head would block
    every chain op behind it).
  - the running KV state is kept block-diagonal [128, 2*65] in bf16 so a
    single matmul (lhsT=UT chunk, K=128 -> FWL) applies BOTH heads' inter
    term, and a single delta matmul (lhsT=ek_tok, rhs=v_aug flat) computes
    both heads' state update (off-diagonal blocks are garbage, never read).
  - token-layout Ek/V (PE transpose + ACT copy) for all four chunks of a
    tile are hoisted to the tile boundary, so the chain's per-chunk
    serial path is just dd [PE] -> state add [DVE] -> inter [PE].
  - finalize(c) is emitted after prep(c+1): the DVE queue is strict FIFO
    and the mask-multiplies gate the next S matmuls, while nothing urgent
    waits on the normalize (od banks are triple-buffered).  The normalize
    is split DVE/ACT per head — except the last chunk, which runs both on
    DVE and stores via ONE [128,128] DMA on the idle sync ring.

HW pitfalls baked in: concurrent (tile_position-overlapped) matmuls must
never write the same PSUM bank (hangs the HW); GPSIMD cannot read PSUM;
DMA-transpose serializes against other DMA traffic (not worth it here).
"""

import os
from contextlib import ExitStack

import numpy as np

import concourse.bass as bass
import concourse.mybir as mybir
import concourse.tile as tile
from concourse import bacc
from concourse._compat import with_exitstack
from concourse.bass import ts

FP32 = mybir.dt.float32
BF16 = mybir.dt.bfloat16

B, N, D, H = 1, 2048, 1024, 16
E = D // H          # 64 head dim
NCORES = 8
HPC = H // NCORES   # 2 heads per core
F = 3 * HPC * E     # 384 per-core projected features (q | k | v)
KT = D // 128       # 8 contraction tiles
TT = 512            # token tile (projection granularity)
NTT = N // TT       # 4
C = 128             # chunk (tokens) for the causal recurrence
CPT = TT // C       # 4 chunks per token tile
NC = N // C         # 16 chunks total
BK = F + TT         # bundle cols per k-tile: [W(384) | x0(512)]
EA = E + 1          # 65: v columns + ones column

Exp = mybir.ActivationFunctionType.Exp
ActCopy = mybir.ActivationFunctionType.Copy
ActIdent = mybir.ActivationFunctionType.Identity
MULT = mybir.AluOpType.mult
ADD = mybir.AluOpType.add


@with_exitstack
def _emit(ctx: ExitStack, tc, io):
    nc = tc.nc
    bund_d, cf_d, xt1_d, xt23_d, out_d = io

    const = ctx.enter_context(tc.tile_pool(name="const", bufs=1))
    chain = ctx.enter_context(tc.tile_pool(name="chain", bufs=2))
    smtp = ctx.enter_context(tc.tile_pool(name="smtp", bufs=2))
    small = ctx.enter_context(tc.tile_pool(name="small", bufs=3))
    outp = ctx.enter_context(tc.tile_pool(name="outp", bufs=3))
    pproj = ctx.enter_context(tc.tile_pool(name="pproj", bufs=2, space="PSUM"))
    # both heads' S^T in one [128, 256] tile (disjoint column halves),
    # double-buffered so chunk c+1's S pair never waits on chunk c's
    # mask-multiply
    ps_scr = ctx.enter_context(tc.tile_pool(name="ps_scr", bufs=1, space="PSUM"))
    # one bank, single-buffered: [tr(Ek) 128 | tr(V) 128] bf16
    ps_tr = ctx.enter_context(tc.tile_pool(name="ps_tr", bufs=1, space="PSUM"))
    # one 2KB bank per chunk: [out (130) | dd (130) | unused]; also hosts the
    # warm-up junk and tile-0's third projection accumulator (V)
    ps_od = ctx.enter_context(tc.tile_pool(name="ps_od", bufs=3, space="PSUM"))

    # ---- persistent SBUF ----
    # bund: [ (W_k(384) | x0_k(512)) * 8 | ident 128 ]
    bund_sb = const.tile([128, KT * BK + 128], BF16)
    cf_sb = const.tile([128, 3 + 2 * C], FP32)  # [bq|bk|bv|mask|mask]
    xtr_sb = const.tile([128, (NTT - 1) * KT * TT], BF16)  # xt tt=1..3, (tt k t)
    kv_st = [
        const.tile([128, HPC * EA], BF16, name=f"kv{i}") for i in range(2)
    ]  # block-diag state

    id_sb = bund_sb[:, KT * BK : KT * BK + 128]
    bq_sb = cf_sb[:, 0:1]
    bk_sb = cf_sb[:, 1:2]
    bv_sb = cf_sb[:, 2:3]  # per-partition (= per V feature) bias column
    mask_sb = cf_sb[:, 3:]  # [128, 256]  two copies of (j, i) 1 iff j<=i

    def w_ap(k, f):
        return bund_sb[:, k * BK + f * 128 : k * BK + (f + 1) * 128]

    def xt_ap(tt, k):
        if tt == 0:
            return bund_sb[:, k * BK + F : k * BK + F + TT]
        base = (tt - 1) * KT * TT + k * TT
        return xtr_sb[:, base : base + TT]

    # ---- input DMA first.  The heavy stream stays on ONE ring (sync) in
    # consumption order; cf rides the scalar (ACT) HWDGE ring in parallel.
    # Each dma_start costs ~600ns of serial descriptor-gen (DIRECT2D) on
    # its ring AND consumers wait on the whole transfer's ~1.7us
    # completion receipt — so: a SMALL first transfer (k0 bundle alone,
    # gates the first real matmul), then k-pairs (halves the issue tail
    # vs per-k while keeping the tile-0 drip fine-grained enough).
    nc.scalar.dma_start(cf_sb[:, :], cf_d[:, :])
    bounds = [0, BK, 3 * BK, 5 * BK, 7 * BK, KT * BK + 128]
    for lo, hi in zip(bounds[:-1], bounds[1:]):
        nc.sync.dma_start(bund_sb[:, lo:hi], bund_d[:, lo:hi])
    nc.sync.dma_start(xtr_sb[:, 0 : KT * TT], xt1_d[:, :])
    nc.sync.dma_start(xtr_sb[:, KT * TT : 2 * KT * TT], xt23_d[:, 0 : KT * TT])
    nc.sync.dma_start(xtr_sb[:, 2 * KT * TT :], xt23_d[:, KT * TT :])

    # zero the off-diagonal blocks of both KV state buffers (they are only
    # ever written in their diagonal blocks)
    nc.gpsimd.memset(kv_st[0][:, :], 0.0)
    nc.gpsimd.memset(kv_st[1][:, :], 0.0)
    junk_sb = const.tile([128, 128], BF16, name="junk_sb")
    nc.gpsimd.memset(junk_sb[:, :], 0.0)
    # wide junk rhs for the HAM warm-up (memset on DVE: it is idle at boot
    # and faster than gpsimd for this size)
    junk2_sb = const.tile([128, 512], BF16, name="junk2_sb")
    nc.vector.memset(junk2_sb[:, :], 0.0)
    # pre-load the ACT exp table NOW (off the critical path) — otherwise
    # the first real EXP pays a ~1.3us ACT_TABLE_LOAD right when the
    # tile-0 projection completes.
    warm_act = const.tile([128, 1], FP32, name="warm_act")
    nc.scalar.activation(warm_act[:, :], junk_sb[:, 0:1], Exp)

    # ---- HAM warm-up: N=512 junk matmuls keep the PE busy from queue
    # start (~7.5us after exec begin) to the k0 bundle's completion sem
    # (~10.8us), so the HAM SHORT window fires (~11.2us) right as the
    # real projection begins instead of ~2us into it.
    junk_ps = ps_od.tile([128, 512], FP32, tag="od", name="junkps")
    for _ in range(7):
        nc.tensor.matmul(
            junk_ps[:, :],
            lhsT=junk_sb[:, :],
            rhs=junk2_sb[:, :],
            start=True,
            stop=True,
        )

    st = [dict(smt=[None] * CPT, ek=[None] * CPT, va=[None] * CPT) for _ in range(NTT)]
    dma_flip = [0]

    _AK = ("UT", "EkT", "VT")

    def emit_act_piece(tt, f, pp, lo, hi, alloc):
        # f == 2 folds the v-bias into V:  sum_j w_ij (v_j + bv) =
        # num_ij + den_i * bv, so out = num'/den needs no bias add.
        s = st[tt]
        if alloc:
            s[_AK[f]] = chain.tile([128, TT], BF16, tag=_AK[f], name=f"{_AK[f]}{tt}")
        dst = s[_AK[f]]
        func = Exp if f < 2 else ActIdent
        nc.scalar.activation(
            dst[:, lo:hi], pp[:, lo:hi], func, bias=cf_sb[:, f : f + 1]
        )

    def emit_act(tt, f, pp):
        # chunk-0 columns first: each tile's chain start gates on the
        # FIRST 128 columns of the exps only, so a small early piece
        # un-gates S / the transposes ~0.4us sooner per tile boundary.
        # V's rest piece is DEFERRED past chunk-0's token-layout copies
        # (it would otherwise sit in front of them in the ACT FIFO and
        # delay the chunk-0 dd by ~0.5us).
        emit_act_piece(tt, f, pp, 0, C, True)
        if f == 2:
            st[tt]["vpp"] = pp
        else:
            emit_act_piece(tt, f, pp, C, TT, False)

    def emit_act_rest_v(tt):
        emit_act_piece(tt, 2, st[tt]["vpp"], C, TT, False)

    def emit_proj_f(tt, f):
        # projection (f-outer): qvk^T[f, t] = sum_d W[d, f] * xT[d, t]
        pp = pproj.tile([128, TT], FP32, tag="proj", name=f"pp{tt}_{f}")
        for k in range(KT):
            nc.tensor.matmul(
                pp[:, :],
                lhsT=w_ap(k, f),
                rhs=xt_ap(tt, k),
                start=(k == 0),
                stop=(k == KT - 1),
            )
        emit_act(tt, f, pp)

    def emit_proj_half(tt, f, hb):
        # half-tile projection (256 tokens) — used to split the LAST tile's
        # projection so half B lands inside its own chain slots, giving the
        # tail real PE work to cover the chain's cross-engine stalls.
        pp = pproj.tile([128, TT], FP32, tag="proj", name=f"pp{tt}_{f}_{hb}")
        lo = hb * 256
        for k in range(KT):
            nc.tensor.matmul(
                pp[:, 0:256],
                lhsT=w_ap(k, f),
                rhs=xt_ap(tt, k)[:, lo : lo + 256],
                start=(k == 0),
                stop=(k == KT - 1),
            )
        s = st[tt]
        if f == 0:
            if hb == 0:
                s["UT"] = chain.tile([128, TT], BF16, tag="UT", name=f"UT{tt}")
            nc.scalar.activation(
                s["UT"][:, lo : lo + 256], pp[:, 0:256], Exp, bias=bq_sb[:, 0:1]
            )
        elif f == 1:
            if hb == 0:
                s["EkT"] = chain.tile([128, TT], BF16, tag="EkT", name=f"EkT{tt}")
            nc.scalar.activation(
                s["EkT"][:, lo : lo + 256], pp[:, 0:256], Exp, bias=bk_sb[:, 0:1]
            )
        else:
            if hb == 0:
                s["VT"] = chain.tile([128, TT], BF16, tag="VT", name=f"VT{tt}")
            nc.scalar.activation(
                s["VT"][:, lo : lo + 256], pp[:, 0:256], ActIdent, bias=bv_sb[:, 0:1]
            )

    def emit_proj_tile0():
        # k-outer / f-inner: each arriving bundle feeds 3 matmuls, PE ramps
        # with the DMA stream and warms HAM on real work.  The third
        # accumulator (V) borrows an od bank so pproj stays at 2 banks.
        pps = [
            pproj.tile([128, TT], FP32, tag="proj", name="pp0_0"),
            pproj.tile([128, TT], FP32, tag="proj", name="pp0_1"),
            ps_od.tile([128, TT], FP32, tag="od", name="pp0_2"),
        ]
        for k in range(KT):
            for f in range(3):
                nc.tensor.matmul(
                    pps[f][:, :],
                    lhsT=w_ap(k, f),
                    rhs=xt_ap(0, k),
                    start=(k == 0),
                    stop=(k == KT - 1),
                )
        # all three chunk-0 exp pieces first, then the rests, so the
        # tile-0 chain start is gated by ~0.6us of ACT instead of ~1.8us
        # (V's rest is deferred past tok(0, 0), like the other tiles)
        for f in range(3):
            emit_act_piece(0, f, pps[f], 0, C, True)
        st[0]["vpp"] = pps[2]
        for f in range(2):
            emit_act_piece(0, f, pps[f], C, TT, False)

    def emit_tok(tt, cc):
        # token-layout Ek / V for one chunk (PE transpose + ACT copy).
        # Emitted for ALL FOUR chunks right at the tile boundary (they
        # only need the tile's exps), so the per-chunk serial path of the
        # chain recurrence is just dd [PE] -> add [DVE] -> inter [PE] —
        # no ACT hop left inside the loop.
        s = st[tt]
        EkT, VT = s["EkT"], s["VT"]
        if cc == 0:
            s["trp"] = ps_tr.tile([128, CPT, 256], BF16, tag="tr", name=f"trp{tt}")
        trp = s["trp"]
        nc.tensor.transpose(trp[:, cc, 0:128], EkT[:, ts(cc, C)], id_sb[:, :])
        nc.tensor.transpose(trp[:, cc, 128:256], VT[:, ts(cc, C)], id_sb[:, :])
        ek_tok = small.tile([128, 128], BF16, tag="ektok", bufs=9, name=f"ek{tt}_{cc}")
        nc.scalar.copy(ek_tok[:, :], trp[:, cc, 0:128])
        v_aug = small.tile(
            [128, HPC, EA], BF16, tag="vaug", bufs=9, name=f"va{tt}_{cc}"
        )
        nc.scalar.copy(
            v_aug[:, :, 0:E],
            trp[:, cc, 128:256].rearrange("p (g e) -> p g e", g=HPC),
        )
        nc.gpsimd.memset(v_aug[:, :, E : E + 1], 1.0)
        s["ek"][cc] = ek_tok
        s["va"][cc] = v_aug

    def emit_prep(tt, cc):
        # per-chunk scores S^T and mask (one chunk ahead of the chain)
        s = st[tt]
        UT, EkT = s["UT"], s["EkT"]
        # S^T[j, i] = sum_d Ek[j,d] U[i,d]  (row-packed head pair, separate
        # PSUM banks so the two matmuls overlap in the array)
        smts = []
        sps_t = []
        for h in range(HPC):
            sps = ps_scr.tile([128, C], FP32, tag=f"s{h}", name=f"sp{tt}_{cc}_{h}")
            nc.tensor.matmul(
                sps[:, :],
                lhsT=EkT[ts(h, E), ts(cc, C)],
                rhs=UT[ts(h, E), ts(cc, C)],
                start=True,
                stop=True,
                tile_position=(E * h, 0),
            )
            sps_t.append(sps)
        for h in range(HPC):
            sm = smtp.tile([128, C], BF16, tag=f"m{h}", bufs=5, name=f"smt{tt}_{cc}_{h}")
            nc.vector.tensor_mul(sm[:, :], sps_t[h][:, :], mask_sb[:, 0:C])
            smts.append(sm)
        s["smt"][cc] = smts

    osb_ref = [None]

    def finalize(out_ps, tt, cc):
        c = tt * CPT + cc
        last_chunk = c == NC - 1
        if cc == 0:
            osb_ref[0] = outp.tile([128, CPT, HPC * E], BF16, tag="osb", name=f"o{tt}")
        osb = osb_ref[0]
        o3 = out_ps.rearrange("p (g e) -> p g e", g=HPC)
        rec = small.tile([128, HPC], FP32, tag="rec", name=f"rec{c}")
        nc.vector.reciprocal(rec[:, :], o3[:, :, E])
        # v-bias is folded into V, so normalize is a pure per-token scale.
        # GpSimd can't read PSUM, so split the two head scales across DVE
        # and ACT to stay under the per-chunk engine cadence — EXCEPT for
        # the last chunk, where the ACT queue is congested with the
        # previous chunk's work: there both scales run back-to-back on
        # DVE and ONE [128, 128] store rides the idle sync ring.
        if last_chunk:
            nc.vector.tensor_mul(
                osb[:, cc, :].rearrange("p (g e) -> p g e", g=HPC),
                o3[:, :, 0:E],
                rec.rearrange("p (g o) -> p g o", o=1).broadcast_to((128, HPC, E)),
            )
            nc.sync.dma_start(out_d[ts(c, C), :], osb[:, cc, :])
            return
        if tt == NTT - 1:
            # tail chunks: DVE's per-chunk FIFO (state adds + mask-mul +
            # reciprocal) IS the chain cadence there, while ACT is idle
            # (the copies were hoisted) — so both scales go to ACT.
            nc.scalar.activation(
                osb[:, cc, ts(0, E)], o3[:, 0, 0:E], ActIdent, scale=rec[:, 0:1]
            )
            nc.scalar.activation(
                osb[:, cc, ts(1, E)], o3[:, 1, 0:E], ActIdent, scale=rec[:, 1:2]
            )
        else:
            # one fused [128, 2, 64] multiply with a broadcast reciprocal
            # instead of two per-head scalar muls (saves a DVE/ACT op +
            # dispatch per chunk)
            nc.vector.tensor_mul(
                osb[:, cc, :].rearrange("p (g e) -> p g e", g=HPC),
                o3[:, :, 0:E],
                rec.rearrange("p (g o) -> p g o", o=1).broadcast_to((128, HPC, E)),
            )
        if tt == NTT - 1:
            # tile-3 flushes alternate the two HWDGE rings (scalar is
            # exp-free by now); gpsimd SWDGE would serialize into the tail
            eng = nc.scalar if dma_flip[0] % 2 else nc.sync
            dma_flip[0] += 1
            eng.dma_start(out_d[ts(c, C), :], osb[:, cc, :])
        elif cc == CPT - 1:
            eng = nc.gpsimd if dma_flip[0] % 2 else nc.sync
            dma_flip[0] += 1
            eng.dma_start(
                out_d[ts(tt, TT), :].rearrange("(cc p) f -> p cc f", p=128),
                osb[:, :, :],
            )

    def emit_chain_chunk(tt, cc):
        s = st[tt]
        UT = s["UT"]
        c = tt * CPT + cc
        smts, ek_tok, v_aug = s["smt"][cc], s["ek"][cc], s["va"][cc]
        vflat = v_aug.rearrange("p g e -> p (g e)")
        od = ps_od.tile([128, 512], FP32, tag="od", name=f"od{c}")
        out_ps = od[:, 0 : HPC * EA]
        # NOTE on start=: start=True clears has_written for the WHOLE bank,
        # so only the FIRST matmul touching this od bank per chunk may set
        # it; later matmuls rely on per-element overwrite-where-unwritten.
        if c < NC - 1:
            # both heads' state delta in one matmul; off-diagonal blocks of
            # dd are cross-head garbage and never read.  Emitted first so
            # the DVE state update has slack before the next chunk's inter.
            dd = od[:, HPC * EA : 2 * HPC * EA]
            nc.tensor.matmul(
                dd[:, :],
                lhsT=ek_tok[:, :],
                rhs=vflat[:, :],
                start=True,
                stop=True,
            )
            nxt = kv_st[c % 2]
            if c == 0:
                nc.vector.tensor_copy(nxt[0:E, 0:EA], dd[0:E, 0:EA])
                nc.vector.tensor_copy(nxt[E:128, EA:], dd[E:128, EA:])
            else:
                prv = kv_st[(c - 1) % 2]
                nc.vector.tensor_add(nxt[0:E, 0:EA], dd[0:E, 0:EA], prv[0:E, 0:EA])
                nc.vector.tensor_add(nxt[E:128, EA:], dd[E:128, EA:], prv[E:128, EA:])

        for h in range(HPC):
            nc.tensor.matmul(
                out_ps[:, h * EA : (h + 1) * EA],
                lhsT=smts[h][:, :],
                rhs=v_aug[:, h, :],
                start=(c == NC - 1 and h == 0),
                stop=(c == 0 and h == HPC - 1),
            )
        if c > 0:
            # both heads' inter term in one K=128 matmul against the
            # block-diagonal bf16 state
            nc.tensor.matmul(
                out_ps[:, :],
                lhsT=UT[:, ts(cc, C)],
                rhs=kv_st[(c - 1) % 2][:, :],
                start=False,
                stop=True,
            )
        return out_ps

    def emit_filler(n, target=None, start=True, wide=False):
        # junk matmuls keep the PE activity monitor from re-throttling
        # across short dependency stalls.  With start=False they may share a
        # live bank's unused columns without clearing its has_written bits.
        if target is None:
            jp = ps_scr.tile([128, C], FP32, tag="s0", name="fill")
            target = jp[0:64, 0:64]
        rhs = junk_sb[:, 0:128] if wide else junk_sb[:, 0:64]
        for _ in range(n):
            nc.tensor.matmul(
                target,
                lhsT=junk_sb[:, 0:64],
                rhs=rhs,
                start=start,
                stop=True,
            )

    # ---- emission: tile-0 projection (k-outer), then ALL of tile-0's
    # chain BEFORE any tile-1 projection matmul: the Tensor queue is
    # strict FIFO, so a tile-1 LDWEIGHTS waiting on the xt1 transfer's
    # completion sem (~20us) at the queue head would block every chain op
    # emitted behind it.  Tiles 1-2 projections are emitted where the
    # input stream has already landed; tile-3 keeps the half-split.
    emit_proj_tile0()
    # bridge the exp/copy latency at the tile-0 -> chain boundary so the
    # activity monitor never sees an idle window there
    emit_filler(4)
    emit_tok(0, 0)
    emit_act_rest_v(0)
    for cc4 in range(1, CPT):
        emit_tok(0, cc4)
    emit_prep(0, 0)
    # finalize(c) is EMITTED after prep(c+1): the DVE queue is strict
    # FIFO, so this lets the mask-multiplies (which gate chunk c+2's S
    # matmuls via WAR on the score banks) run before chunk c's
    # reciprocal/normalize, which nothing urgent waits on (od bufs=3).
    pending = [None]

    def flush_pending():
        if pending[0] is not None:
            finalize(*pending[0])
            pending[0] = None

    for tt in range(NTT):
        if tt == 1:
            slices = [lambda: emit_proj_f(2, 1), lambda: emit_proj_f(2, 2), None]
        elif tt == 2:
            slices = [lambda f=f: emit_proj_f(3, f) for f in range(3)]
        else:
            slices = [None, None, None]
        for cc in range(CPT):
            out_ps = emit_chain_chunk(tt, cc)
            if tt == NTT - 1 and cc == CPT - 1:
                finalize(out_ps, tt, cc)
                break
            pending[0] = (out_ps, tt, cc)
            if cc < CPT - 1:
                emit_prep(tt, cc + 1)
                if slices[cc] is not None:
                    slices[cc]()
                flush_pending()
        if tt == 0:
            # tile-1 projection (and tile-2's f0) go here: by the time the
            # PE drains the tile-0 chain, xt1's sem has fired, so these
            # never block the queue; f(2,0) covers the tile-1 exp latency.
            for f1 in range(3):
                emit_proj_f(1, f1)
            emit_proj_f(2, 0)
        if tt < NTT - 1:
            # chunk-0's token-layout tensors first, THEN V's deferred
            # rest-exp, then the remaining chunks' toks: keeps chunk 0's
            # dd un-gated by the [128, 384] V exp in the ACT FIFO
            emit_tok(tt + 1, 0)
            emit_act_rest_v(tt + 1)
            for cc4 in range(1, CPT):
                emit_tok(tt + 1, cc4)
            emit_prep(tt + 1, 0)
            flush_pending()


def build_nc():
    nc = bacc.Bacc(
        "TRN2",
        target_bir_lowering=False,
        debug=False,
        enable_asserts=False,
        num_devices=NCORES,
    )
    bund_d = nc.dram_tensor(
        "bund", [128, KT * BK + 128], BF16, kind="ExternalInput"
    ).ap()
    cf_d = nc.dram_tensor("cf", [128, 3 + 2 * C], FP32, kind="ExternalInput").ap()
    xt1_d = nc.dram_tensor("xt1", [128, KT * TT], BF16, kind="ExternalInput").ap()
    xt23_d = nc.dram_tensor(
        "xt23", [128, 2 * KT * TT], BF16, kind="ExternalInput"
    ).ap()
    out_d = nc.dram_tensor("out", [N, HPC * E], BF16, kind="ExternalOutput").ap()
    io = (bund_d, cf_d, xt1_d, xt23_d, out_d)
    with tile.TileContext(nc) as tc:
        _emit(tc, io)
    nc.compile()
    return nc


def host_inputs(x, W_qvk, b_qvk):
    """Full inputs -> per-core in_maps (host-side shard + transpose)."""
    import ml_dtypes

    x = np.asarray(x, dtype=np.float32).reshape(N, D)
    W = np.asarray(W_qvk, dtype=np.float32)
    b = np.asarray(b_qvk, dtype=np.float32)
    xt = x.T.astype(ml_dtypes.bfloat16)  # (D, N)

    def pack(a):  # (D, M) -> (128, KT*M), partition-contiguous
        kt, m = a.shape[0] // 128, a.shape[1]
        return np.ascontiguousarray(
            a.reshape(kt, 128, m).transpose(1, 0, 2).reshape(128, kt * m)
        )

    xtp = [pack(xt[:, tt * TT : (tt + 1) * TT]) for tt in range(NTT)]
    xt1 = xtp[1]
    xt23 = np.ascontiguousarray(np.concatenate([xtp[2], xtp[3]], axis=1))
    ident = np.eye(128, dtype=ml_dtypes.bfloat16)

    tri = np.tril(np.ones((C, C), dtype=np.float32))  # [i, j] valid j<=i
    mask = np.ascontiguousarray(tri.T)  # [j, i] 1 iff j<=i

    in_maps = []
    for core in range(NCORES):
        heads = [HPC * core + i for i in range(HPC)]
        # torch.chunk order in reference: q, v, k
        qcols = np.concatenate([np.arange(E * h, E * h + E) for h in heads])
        vcols = qcols + D
        kcols = qcols + 2 * D
        Wc = pack(
            np.concatenate([W[:, qcols], W[:, kcols], W[:, vcols]], axis=1).astype(
                ml_dtypes.bfloat16
            )
        )
        # bundle layout per k: [W_k (384) | x0_k (512)], then ident
        bund = np.empty((128, KT * BK + 128), dtype=ml_dtypes.bfloat16)
        for k in range(KT):
            bund[:, k * BK : k * BK + F] = Wc[:, k * F : (k + 1) * F]
            bund[:, k * BK + F : (k + 1) * BK] = xtp[0][:, k * TT : (k + 1) * TT]
        bund[:, KT * BK :] = ident
        bq = b[qcols].reshape(128, 1)
        bk = b[kcols].reshape(128, 1)
        bv = b[vcols].reshape(128, 1)
        cf = np.ascontiguousarray(
            np.concatenate([bq, bk, bv, mask, mask], axis=1, dtype=np.float32)
        )
        in_maps.append(dict(bund=bund, cf=cf, xt1=xt1, xt23=xt23))
    return in_maps


_CACHE = {}


def kernel(x, W_qvk, b_qvk, head_num):
    assert int(np.asarray(head_num)) == H
    if "nc" not in _CACHE:
        _CACHE["nc"] = build_nc()
    nc = _CACHE["nc"]
    in_maps = host_inputs(x, W_qvk, b_qvk)
    from concourse.bass_utils import run_bass_kernel_spmd

    res = run_bass_kernel_spmd(
        nc,
        in_maps,
        core_ids=list(range(NCORES)),
        trace=bool(int(os.environ.get("KERNEL_TRACE", "0"))),
    )
    _CACHE["last_result"] = res
    out = np.concatenate(
        [np.asarray(r["out"], dtype=np.float32) for r in res.results], axis=1
    )
    return out.reshape(B, N, D)



# revision 78
# speedup vs baseline: 1.0964x; 1.0964x over previous
"""Multi-head causal linear attention (B=1, N=2048, D=1024, H=16) on 8 trn2 cores.

Math: reference computes, per head (e=64):
    q = softmax(q_raw, -1) * e**-0.5 ;  k = exp(k_raw)
    out_n = (q_n . KV_n) / (q_n . (kcum_n + EPS)),  KV_n = sum_{j<=n} k_j v_j^T
Because both numerator and denominator are linear in q_n, the softmax
normalization and the e**-0.5 scale cancel exactly; only u = exp(q_raw)
matters.  The EPS term contributes <1e-6 relative and is dropped.  The
v-bias contribution factors out:  out += b_v  (sum_j s_nj / denom ~= 1).

Per-core work (head-parallel, 2 heads/core):
    qvk^T = W_c^T @ x  computed as matmul(lhsT=W_block, rhs=x^T) on PE,
    x^T is pre-transposed on the host so no on-chip transpose is needed.
    Chunked causal linear attention (chunk=128) with the classic
    intra (masked QK^T V) + inter (running KV state) recurrence.

Layout tricks:
  - input DMA is issued first: cf on the scalar HWDGE ring, the heavy
    stream on sync in consumption order as [k0 bundle] + k-pairs + xt
    tiles.  Each dma_start costs ~600ns of serial DIRECT2D descriptor-gen
    on its ring, and consumers gate on the transfer's completion sem
    (last byte + ~1-2us receipt), so the first transfer is small and the
    rest are paired.
  - N=512 junk matmuls bridge queue-start (~7.5us of runtime boot) to the
    k0 sem so the HAM clock is warm when real work begins; the ACT exp
    table is pre-loaded off the critical path.
  - the first token tile's projection runs k-outer / f-inner so the PE
    starts as soon as bundle 0 lands; ALL of tile-0's chain is emitted
    before any tile-1 projection matmul (the Tensor queue is strict FIFO
    — an LDWEIGHTS waiting on xt1's sem at the queue head would block
    every chain op behind it).
  - the running KV state is kept block-diagonal [128, 2*65] in bf16 so a
    single matmul (lhsT=UT chunk, K=128 -> FWL) applies BOTH heads' inter
    term, and a single delta matmul (lhsT=ek_tok, rhs=v_aug flat) computes
    both heads' state update (off-diagonal blocks are garbage, never read).
  - token-layout Ek/V (PE transpose + ACT copy) for all four chunks of a
    tile are hoisted to the tile boundary, so the chain's per-chunk
    serial path is just dd [PE] -> state add [DVE] -> inter [PE].
  - finalize(c) is emitted after prep(c+1): the DVE queue is strict FIFO
    and the mask-multiplies gate the next S matmuls, while nothing urgent
    waits on the normalize (od banks are triple-buffered).  The normalize
    is split DVE/ACT per head — except the last chunk, which runs both on
    DVE and stores via ONE [128,128] DMA on the idle sync ring.

HW pitfalls baked in: concurrent (tile_position-overlapped) matmuls must
never write the same PSUM bank (hangs the HW); GPSIMD cannot read PSUM;
DMA-transpose serializes against other DMA traffic (not worth it here).
"""

import os
from contextlib import ExitStack

import numpy as np

import concourse.bass as bass
import concourse.mybir as mybir
import concourse.tile as tile
from concourse import bacc
from concourse._compat import with_exitstack
from concourse.bass import ts

FP32 = mybir.dt.float32
BF16 = mybir.dt.bfloat16

B, N, D, H = 1, 2048, 1024, 16
E = D // H          # 64 head dim
NCORES = 8
HPC = H // NCORES   # 2 heads per core
F = 3 * HPC * E     # 384 per-core projected features (q | k | v)
KT = D // 128       # 8 contraction tiles
TT = 512            # token tile (projection granularity)
NTT = N // TT       # 4
C = 128             # chunk (tokens) for the causal recurrence
CPT = TT // C       # 4 chunks per token tile
NC = N // C         # 16 chunks total
BK = F + TT         # bundle cols per k-tile: [W(384) | x0(512)]
EA = E + 1          # 65: v columns + ones column

Exp = mybir.ActivationFunctionType.Exp
ActCopy = mybir.ActivationFunctionType.Copy
ActIdent = mybir.ActivationFunctionType.Identity
MULT = mybir.AluOpType.mult
ADD = mybir.AluOpType.add


@with_exitstack
def _emit(ctx: ExitStack, tc, io):
    nc = tc.nc
    bund_d, cf_d, xt1_d, xt23_d, out_d = io

    const = ctx.enter_context(tc.tile_pool(name="const", bufs=1))
    chain = ctx.enter_context(tc.tile_pool(name="chain", bufs=2))
    smtp = ctx.enter_context(tc.tile_pool(name="smtp", bufs=2))
    small = ctx.enter_context(tc.tile_pool(name="small", bufs=3))
    outp = ctx.enter_context(tc.tile_pool(name="outp", bufs=3))
    pproj = ctx.enter_context(tc.tile_pool(name="pproj", bufs=2, space="PSUM"))
    # both heads' S^T in one [128, 256] tile (disjoint column halves),
    # double-buffered so chunk c+1's S pair never waits on chunk c's
    # mask-multiply
    ps_scr = ctx.enter_context(tc.tile_pool(name="ps_scr", bufs=1, space="PSUM"))
    # one bank, single-buffered: [tr(Ek) 128 | tr(V) 128] bf16
    ps_tr = ctx.enter_context(tc.tile_pool(name="ps_tr", bufs=1, space="PSUM"))
    # one 2KB bank per chunk: [out (130) | dd (130) | unused]; also hosts the
    # warm-up junk and tile-0's third projection accumulator (V)
    ps_od = ctx.enter_context(tc.tile_pool(name="ps_od", bufs=3, space="PSUM"))

    # ---- persistent SBUF ----
    # bund: [ (W_k(384) | x0_k(512)) * 8 | ident 128 ]
    bund_sb = const.tile([128, KT * BK + 128], BF16)
    cf_sb = const.tile([128, 3 + 2 * C], FP32)  # [bq|bk|bv|mask|mask]
    xtr_sb = const.tile([128, (NTT - 1) * KT * TT], BF16)  # xt tt=1..3, (tt k t)
    kv_st = [
        const.tile([128, HPC * EA], BF16, name=f"kv{i}") for i in range(2)
    ]  # block-diag state

    id_sb = bund_sb[:, KT * BK : KT * BK + 128]
    bq_sb = cf_sb[:, 0:1]
    bk_sb = cf_sb[:, 1:2]
    bv_sb = cf_sb[:, 2:3]  # per-partition (= per V feature) bias column
    mask_sb = cf_sb[:, 3:]  # [128, 256]  two copies of (j, i) 1 iff j<=i

    def w_ap(k, f):
        return bund_sb[:, k * BK + f * 128 : k * BK + (f + 1) * 128]

    def xt_ap(tt, k):
        if tt == 0:
            return bund_sb[:, k * BK + F : k * BK + F + TT]
        base = (tt - 1) * KT * TT + k * TT
        return xtr_sb[:, base : base + TT]

    # ---- input DMA first.  The heavy stream stays on ONE ring (sync) in
    # consumption order; cf rides the scalar (ACT) HWDGE ring in parallel.
    # Each dma_start costs ~600ns of serial descriptor-gen (DIRECT2D) on
    # its ring AND consumers wait on the whole transfer's ~1.7us
    # completion receipt — so: a SMALL first transfer (k0 bundle alone,
    # gates the first real matmul), then k-pairs (halves the issue tail
    # vs per-k while keeping the tile-0 drip fine-grained enough).
    nc.scalar.dma_start(cf_sb[:, :], cf_d[:, :])
    # k3 gets its own transfer: its completion sem gated a ~1.5us PE
    # stall in tile-0 that propagates to the chain start; the extra
    # descriptor-gen slot still finishes before the drain engines reach
    # the later transfers, so nothing downstream moves.
    bounds = [0, BK, 3 * BK, 4 * BK, 5 * BK, 7 * BK, KT * BK + 128]
    for lo, hi in zip(bounds[:-1], bounds[1:]):
        nc.sync.dma_start(bund_sb[:, lo:hi], bund_d[:, lo:hi])
    nc.sync.dma_start(xtr_sb[:, 0 : KT * TT], xt1_d[:, :])
    nc.sync.dma_start(xtr_sb[:, KT * TT : 2 * KT * TT], xt23_d[:, 0 : KT * TT])
    nc.sync.dma_start(xtr_sb[:, 2 * KT * TT :], xt23_d[:, KT * TT :])

    # zero the off-diagonal blocks of both KV state buffers (they are only
    # ever written in their diagonal blocks)
    nc.gpsimd.memset(kv_st[0][:, :], 0.0)
    nc.gpsimd.memset(kv_st[1][:, :], 0.0)
    junk_sb = const.tile([128, 128], BF16, name="junk_sb")
    nc.gpsimd.memset(junk_sb[:, :], 0.0)
    # wide junk rhs for the HAM warm-up (memset on DVE: it is idle at boot
    # and faster than gpsimd for this size)
    junk2_sb = const.tile([128, 512], BF16, name="junk2_sb")
    nc.vector.memset(junk2_sb[:, :], 0.0)
    # pre-load the ACT exp table NOW (off the critical path) — otherwise
    # the first real EXP pays a ~1.3us ACT_TABLE_LOAD right when the
    # tile-0 projection completes.
    warm_act = const.tile([128, 1], FP32, name="warm_act")
    nc.scalar.activation(warm_act[:, :], junk_sb[:, 0:1], Exp)

    # ---- HAM warm-up: N=512 junk matmuls keep the PE busy from queue
    # start (~7.5us after exec begin) to the k0 bundle's completion sem
    # (~10.8us), so the HAM SHORT window fires (~11.2us) right as the
    # real projection begins instead of ~2us into it.
    junk_ps = ps_od.tile([128, 512], FP32, tag="od", name="junkps")
    for _ in range(7):
        nc.tensor.matmul(
            junk_ps[:, :],
            lhsT=junk_sb[:, :],
            rhs=junk2_sb[:, :],
            start=True,
            stop=True,
        )

    st = [dict(smt=[None] * CPT, ek=[None] * CPT, va=[None] * CPT) for _ in range(NTT)]
    dma_flip = [0]

    _AK = ("UT", "EkT", "VT")

    def emit_act_piece(tt, f, pp, lo, hi, alloc):
        # f == 2 folds the v-bias into V:  sum_j w_ij (v_j + bv) =
        # num_ij + den_i * bv, so out = num'/den needs no bias add.
        s = st[tt]
        if alloc:
            s[_AK[f]] = chain.tile([128, TT], BF16, tag=_AK[f], name=f"{_AK[f]}{tt}")
        dst = s[_AK[f]]
        func = Exp if f < 2 else ActIdent
        nc.scalar.activation(
            dst[:, lo:hi], pp[:, lo:hi], func, bias=cf_sb[:, f : f + 1]
        )

    def emit_act(tt, f, pp):
        # chunk-0 columns first: each tile's chain start gates on the
        # FIRST 128 columns of the exps only, so a small early piece
        # un-gates S / the transposes ~0.4us sooner per tile boundary.
        # V's rest piece is DEFERRED past chunk-0's token-layout copies
        # (it would otherwise sit in front of them in the ACT FIFO and
        # delay the chunk-0 dd by ~0.5us).
        emit_act_piece(tt, f, pp, 0, C, True)
        if f == 2:
            st[tt]["vpp"] = pp
        else:
            emit_act_piece(tt, f, pp, C, TT, False)

    def emit_act_rest_v(tt):
        emit_act_piece(tt, 2, st[tt]["vpp"], C, TT, False)

    def emit_proj_f(tt, f):
        # projection (f-outer): qvk^T[f, t] = sum_d W[d, f] * xT[d, t]
        pp = pproj.tile([128, TT], FP32, tag="proj", name=f"pp{tt}_{f}")
        for k in range(KT):
            nc.tensor.matmul(
                pp[:, :],
                lhsT=w_ap(k, f),
                rhs=xt_ap(tt, k),
                start=(k == 0),
                stop=(k == KT - 1),
            )
        emit_act(tt, f, pp)

    def emit_proj_half(tt, f, hb):
        # half-tile projection (256 tokens) — used to split the LAST tile's
        # projection so half B lands inside its own chain slots, giving the
        # tail real PE work to cover the chain's cross-engine stalls.
        pp = pproj.tile([128, TT], FP32, tag="proj", name=f"pp{tt}_{f}_{hb}")
        lo = hb * 256
        for k in range(KT):
            nc.tensor.matmul(
                pp[:, 0:256],
                lhsT=w_ap(k, f),
                rhs=xt_ap(tt, k)[:, lo : lo + 256],
                start=(k == 0),
                stop=(k == KT - 1),
            )
        s = st[tt]
        if f == 0:
            if hb == 0:
                s["UT"] = chain.tile([128, TT], BF16, tag="UT", name=f"UT{tt}")
            nc.scalar.activation(
                s["UT"][:, lo : lo + 256], pp[:, 0:256], Exp, bias=bq_sb[:, 0:1]
            )
        elif f == 1:
            if hb == 0:
                s["EkT"] = chain.tile([128, TT], BF16, tag="EkT", name=f"EkT{tt}")
            nc.scalar.activation(
                s["EkT"][:, lo : lo + 256], pp[:, 0:256], Exp, bias=bk_sb[:, 0:1]
            )
        else:
            if hb == 0:
                s["VT"] = chain.tile([128, TT], BF16, tag="VT", name=f"VT{tt}")
            nc.scalar.activation(
                s["VT"][:, lo : lo + 256], pp[:, 0:256], ActIdent, bias=bv_sb[:, 0:1]
            )

    def emit_proj_tile0():
        # k-outer / f-inner: each arriving bundle feeds 3 matmuls, PE ramps
        # with the DMA stream and warms HAM on real work.  The third
        # accumulator (V) borrows an od bank so pproj stays at 2 banks.
        pps = [
            pproj.tile([128, TT], FP32, tag="proj", name="pp0_0"),
            pproj.tile([128, TT], FP32, tag="proj", name="pp0_1"),
            ps_od.tile([128, TT], FP32, tag="od", name="pp0_2"),
        ]
        for k in range(KT):
            for f in range(3):
                nc.tensor.matmul(
                    pps[f][:, :],
                    lhsT=w_ap(k, f),
                    rhs=xt_ap(0, k),
                    start=(k == 0),
                    stop=(k == KT - 1),
                )
        # all three chunk-0 exp pieces first, then the rests, so the
        # tile-0 chain start is gated by ~0.6us of ACT instead of ~1.8us
        # (V's rest is deferred past tok(0, 0), like the other tiles)
        for f in range(3):
            emit_act_piece(0, f, pps[f], 0, C, True)
        st[0]["vpp"] = pps[2]
        for f in range(2):
            emit_act_piece(0, f, pps[f], C, TT, False)

    def emit_tok(tt, cc):
        # token-layout Ek / V for one chunk (PE transpose + ACT copy).
        # Emitted for ALL FOUR chunks right at the tile boundary (they
        # only need the tile's exps), so the per-chunk serial path of the
        # chain recurrence is just dd [PE] -> add [DVE] -> inter [PE] —
        # no ACT hop left inside the loop.
        s = st[tt]
        EkT, VT = s["EkT"], s["VT"]
        if cc == 0:
            s["trp"] = ps_tr.tile([128, CPT, 256], BF16, tag="tr", name=f"trp{tt}")
        trp = s["trp"]
        nc.tensor.transpose(trp[:, cc, 0:128], EkT[:, ts(cc, C)], id_sb[:, :])
        nc.tensor.transpose(trp[:, cc, 128:256], VT[:, ts(cc, C)], id_sb[:, :])
        ek_tok = small.tile([128, 128], BF16, tag="ektok", bufs=9, name=f"ek{tt}_{cc}")
        nc.scalar.copy(ek_tok[:, :], trp[:, cc, 0:128])
        v_aug = small.tile(
            [128, HPC, EA], BF16, tag="vaug", bufs=9, name=f"va{tt}_{cc}"
        )
        nc.scalar.copy(
            v_aug[:, :, 0:E],
            trp[:, cc, 128:256].rearrange("p (g e) -> p g e", g=HPC),
        )
        nc.gpsimd.memset(v_aug[:, :, E : E + 1], 1.0)
        s["ek"][cc] = ek_tok
        s["va"][cc] = v_aug

    def emit_prep(tt, cc):
        # per-chunk scores S^T and mask (one chunk ahead of the chain)
        s = st[tt]
        UT, EkT = s["UT"], s["EkT"]
        # S^T[j, i] = sum_d Ek[j,d] U[i,d]  (row-packed head pair, separate
        # PSUM banks so the two matmuls overlap in the array)
        smts = []
        sps_t = []
        for h in range(HPC):
            sps = ps_scr.tile([128, C], FP32, tag=f"s{h}", name=f"sp{tt}_{cc}_{h}")
            nc.tensor.matmul(
                sps[:, :],
                lhsT=EkT[ts(h, E), ts(cc, C)],
                rhs=UT[ts(h, E), ts(cc, C)],
                start=True,
                stop=True,
                tile_position=(E * h, 0),
            )
            sps_t.append(sps)
        for h in range(HPC):
            sm = smtp.tile([128, C], BF16, tag=f"m{h}", bufs=5, name=f"smt{tt}_{cc}_{h}")
            nc.vector.tensor_mul(sm[:, :], sps_t[h][:, :], mask_sb[:, 0:C])
            smts.append(sm)
        s["smt"][cc] = smts

    osb_ref = [None]

    def finalize(out_ps, tt, cc):
        c = tt * CPT + cc
        last_chunk = c == NC - 1
        if cc == 0:
            osb_ref[0] = outp.tile([128, CPT, HPC * E], BF16, tag="osb", name=f"o{tt}")
        osb = osb_ref[0]
        o3 = out_ps.rearrange("p (g e) -> p g e", g=HPC)
        rec = small.tile([128, HPC], FP32, tag="rec", name=f"rec{c}")
        nc.vector.reciprocal(rec[:, :], o3[:, :, E])
        # v-bias is folded into V, so normalize is a pure per-token scale.
        # GpSimd can't read PSUM, so split the two head scales across DVE
        # and ACT to stay under the per-chunk engine cadence — EXCEPT for
        # the last chunk, where the ACT queue is congested with the
        # previous chunk's work: there both scales run back-to-back on
        # DVE and ONE [128, 128] store rides the idle sync ring.
        if last_chunk:
            nc.vector.tensor_mul(
                osb[:, cc, :].rearrange("p (g e) -> p g e", g=HPC),
                o3[:, :, 0:E],
                rec.rearrange("p (g o) -> p g o", o=1).broadcast_to((128, HPC, E)),
            )
            nc.sync.dma_start(out_d[ts(c, C), :], osb[:, cc, :])
            return
        if tt == NTT - 1:
            # tail chunks: DVE's per-chunk FIFO (state adds + mask-mul +
            # reciprocal) IS the chain cadence there, while ACT is idle
            # (the copies were hoisted) — so both scales go to ACT.
            nc.scalar.activation(
                osb[:, cc, ts(0, E)], o3[:, 0, 0:E], ActIdent, scale=rec[:, 0:1]
            )
            nc.scalar.activation(
                osb[:, cc, ts(1, E)], o3[:, 1, 0:E], ActIdent, scale=rec[:, 1:2]
            )
        else:
            # one fused [128, 2, 64] multiply with a broadcast reciprocal
            # instead of two per-head scalar muls (saves a DVE/ACT op +
            # dispatch per chunk)
            nc.vector.tensor_mul(
                osb[:, cc, :].rearrange("p (g e) -> p g e", g=HPC),
                o3[:, :, 0:E],
                rec.rearrange("p (g o) -> p g o", o=1).broadcast_to((128, HPC, E)),
            )
        if tt == NTT - 1:
            # tile-3 flushes alternate the two HWDGE rings (scalar is
            # exp-free by now); gpsimd SWDGE would serialize into the tail
            eng = nc.scalar if dma_flip[0] % 2 else nc.sync
            dma_flip[0] += 1
            eng.dma_start(out_d[ts(c, C), :], osb[:, cc, :])
        elif cc == CPT - 1:
            eng = nc.gpsimd if dma_flip[0] % 2 else nc.sync
            dma_flip[0] += 1
            eng.dma_start(
                out_d[ts(tt, TT), :].rearrange("(cc p) f -> p cc f", p=128),
                osb[:, :, :],
            )

    def emit_chain_chunk(tt, cc):
        s = st[tt]
        UT = s["UT"]
        c = tt * CPT + cc
        smts, ek_tok, v_aug = s["smt"][cc], s["ek"][cc], s["va"][cc]
        vflat = v_aug.rearrange("p g e -> p (g e)")
        od = ps_od.tile([128, 512], FP32, tag="od", name=f"od{c}")
        out_ps = od[:, 0 : HPC * EA]
        # NOTE on start=: start=True clears has_written for the WHOLE bank,
        # so only the FIRST matmul touching this od bank per chunk may set
        # it; later matmuls rely on per-element overwrite-where-unwritten.
        if c < NC - 1:
            # both heads' state delta in one matmul; off-diagonal blocks of
            # dd are cross-head garbage and never read.  Emitted first so
            # the DVE state update has slack before the next chunk's inter.
            dd = od[:, HPC * EA : 2 * HPC * EA]
            nc.tensor.matmul(
                dd[:, :],
                lhsT=ek_tok[:, :],
                rhs=vflat[:, :],
                start=True,
                stop=True,
            )
            nxt = kv_st[c % 2]
            if c == 0:
                nc.vector.tensor_copy(nxt[0:E, 0:EA], dd[0:E, 0:EA])
                nc.vector.tensor_copy(nxt[E:128, EA:], dd[E:128, EA:])
            else:
                prv = kv_st[(c - 1) % 2]
                nc.vector.tensor_add(nxt[0:E, 0:EA], dd[0:E, 0:EA], prv[0:E, 0:EA])
                nc.vector.tensor_add(nxt[E:128, EA:], dd[E:128, EA:], prv[E:128, EA:])

        for h in range(HPC):
            nc.tensor.matmul(
                out_ps[:, h * EA : (h + 1) * EA],
                lhsT=smts[h][:, :],
                rhs=v_aug[:, h, :],
                start=(c == NC - 1 and h == 0),
                stop=(c == 0 and h == HPC - 1),
            )
        if c > 0:
            # both heads' inter term in one K=128 matmul against the
            # block-diagonal bf16 state
            nc.tensor.matmul(
                out_ps[:, :],
                lhsT=UT[:, ts(cc, C)],
                rhs=kv_st[(c - 1) % 2][:, :],
                start=False,
                stop=True,
            )
        return out_ps

    def emit_filler(n, target=None, start=True, wide=False):
        # junk matmuls keep the PE activity monitor from re-throttling
        # across short dependency stalls.  With start=False they may share a
        # live bank's unused columns without clearing its has_written bits.
        if target is None:
            jp = ps_scr.tile([128, C], FP32, tag="s0", name="fill")
            target = jp[0:64, 0:64]
        rhs = junk_sb[:, 0:128] if wide else junk_sb[:, 0:64]
        for _ in range(n):
            nc.tensor.matmul(
                target,
                lhsT=junk_sb[:, 0:64],
                rhs=rhs,
                start=start,
                stop=True,
            )

    # ---- emission: tile-0 projection (k-outer), then ALL of tile-0's
    # chain BEFORE any tile-1 projection matmul: the Tensor queue is
    # strict FIFO, so a tile-1 LDWEIGHTS waiting on the xt1 transfer's
    # completion sem (~20us) at the queue head would block every chain op
    # emitted behind it.  Tiles 1-2 projections are emitted where the
    # input stream has already landed; tile-3 keeps the half-split.
    emit_proj_tile0()
    # bridge the exp/copy latency at the tile-0 -> chain boundary so the
    # activity monitor never sees an idle window there
    emit_filler(4)
    emit_tok(0, 0)
    emit_act_rest_v(0)
    for cc4 in range(1, CPT):
        emit_tok(0, cc4)
    emit_prep(0, 0)
    # finalize(c) is EMITTED after prep(c+1): the DVE queue is strict
    # FIFO, so this lets the mask-multiplies (which gate chunk c+2's S
    # matmuls via WAR on the score banks) run before chunk c's
    # reciprocal/normalize, which nothing urgent waits on (od bufs=3).
    pending = [None]

    def flush_pending():
        if pending[0] is not None:
            finalize(*pending[0])
            pending[0] = None

    for tt in range(NTT):
        if tt == 1:
            slices = [lambda: emit_proj_f(2, 1), lambda: emit_proj_f(2, 2), None]
        elif tt == 2:
            slices = [lambda f=f: emit_proj_f(3, f) for f in range(3)]
        else:
            slices = [None, None, None]
        for cc in range(CPT):
            out_ps = emit_chain_chunk(tt, cc)
            if tt == NTT - 1 and cc == CPT - 1:
                finalize(out_ps, tt, cc)
                break
            pending[0] = (out_ps, tt, cc)
            if cc < CPT - 1:
                emit_prep(tt, cc + 1)
                if slices[cc] is not None:
                    slices[cc]()
                flush_pending()
        if tt == 0:
            # tile-1 projection (and tile-2's f0) go here: by the time the
            # PE drains the tile-0 chain, xt1's sem has fired, so these
            # never block the queue; f(2,0) covers the tile-1 exp latency.
            for f1 in range(3):
                emit_proj_f(1, f1)
            emit_proj_f(2, 0)
        if tt < NTT - 1:
            # chunk-0's token-layout tensors first, THEN V's deferred
            # rest-exp, then the remaining chunks' toks: keeps chunk 0's
            # dd un-gated by the [128, 384] V exp in the ACT FIFO
            emit_tok(tt + 1, 0)
            emit_act_rest_v(tt + 1)
            for cc4 in range(1, CPT):
                emit_tok(tt + 1, cc4)
            emit_prep(tt + 1, 0)
            flush_pending()


def build_nc():
    nc = bacc.Bacc(
        "TRN2",
        target_bir_lowering=False,
        debug=False,
        enable_asserts=False,
        num_devices=NCORES,
    )
    bund_d = nc.dram_tensor(
        "bund", [128, KT * BK + 128], BF16, kind="ExternalInput"
    ).ap()
    cf_d = nc.dram_tensor("cf", [128, 3 + 2 * C], FP32, kind="ExternalInput").ap()
    xt1_d = nc.dram_tensor("xt1", [128, KT * TT], BF16, kind="ExternalInput").ap()
    xt23_d = nc.dram_tensor(
        "xt23", [128, 2 * KT * TT], BF16, kind="ExternalInput"
    ).ap()
    out_d = nc.dram_tensor("out", [N, HPC * E], BF16, kind="ExternalOutput").ap()
    io = (bund_d, cf_d, xt1_d, xt23_d, out_d)
    with tile.TileContext(nc) as tc:
        _emit(tc, io)
    nc.compile()
    return nc


def host_inputs(x, W_qvk, b_qvk):
    """Full inputs -> per-core in_maps (host-side shard + transpose)."""
    import ml_dtypes

    x = np.asarray(x, dtype=np.float32).reshape(N, D)
    W = np.asarray(W_qvk, dtype=np.float32)
    b = np.asarray(b_qvk, dtype=np.float32)
    xt = x.T.astype(ml_dtypes.bfloat16)  # (D, N)

    def pack(a):  # (D, M) -> (128, KT*M), partition-contiguous
        kt, m = a.shape[0] // 128, a.shape[1]
        return np.ascontiguousarray(
            a.reshape(kt, 128, m).transpose(1, 0, 2).reshape(128, kt * m)
        )

    xtp = [pack(xt[:, tt * TT : (tt + 1) * TT]) for tt in range(NTT)]
    xt1 = xtp[1]
    xt23 = np.ascontiguousarray(np.concatenate([xtp[2], xtp[3]], axis=1))
    ident = np.eye(128, dtype=ml_dtypes.bfloat16)

    tri = np.tril(np.ones((C, C), dtype=np.float32))  # [i, j] valid j<=i
    mask = np.ascontiguousarray(tri.T)  # [j, i] 1 iff j<=i

    in_maps = []
    for core in range(NCORES):
        heads = [HPC * core + i for i in range(HPC)]
        # torch.chunk order in reference: q, v, k
        qcols = np.concatenate([np.arange(E * h, E * h + E) for h in heads])
        vcols = qcols + D
        kcols = qcols + 2 * D
        Wc = pack(
            np.concatenate([W[:, qcols], W[:, kcols], W[:, vcols]], axis=1).astype(
                ml_dtypes.bfloat16
            )
        )
        # bundle layout per k: [W_k (384) | x0_k (512)], then ident
        bund = np.empty((128, KT * BK + 128), dtype=ml_dtypes.bfloat16)
        for k in range(KT):
            bund[:, k * BK : k * BK + F] = Wc[:, k * F : (k + 1) * F]
            bund[:, k * BK + F : (k + 1) * BK] = xtp[0][:, k * TT : (k + 1) * TT]
        bund[:, KT * BK :] = ident
        bq = b[qcols].reshape(128, 1)
        bk = b[kcols].reshape(128, 1)
        bv = b[vcols].reshape(128, 1)
        cf = np.ascontiguousarray(
            np.concatenate([bq, bk, bv, mask, mask], axis=1, dtype=np.float32)
        )
        in_maps.append(dict(bund=bund, cf=cf, xt1=xt1, xt23=xt23))
    return in_maps


_CACHE = {}


def kernel(x, W_qvk, b_qvk, head_num):
    assert int(np.asarray(head_num)) == H
    if "nc" not in _CACHE:
        _CACHE["nc"] = build_nc()
    nc = _CACHE["nc"]
    in_maps = host_inputs(x, W_qvk, b_qvk)
    from concourse.bass_utils import run_bass_kernel_spmd

    res = run_bass_kernel_spmd(
        nc,
        in_maps,
        core_ids=list(range(NCORES)),
        trace=bool(int(os.environ.get("KERNEL_TRACE", "0"))),
    )
    _CACHE["last_result"] = res
    out = np.concatenate(
        [np.asarray(r["out"], dtype=np.float32) for r in res.results], axis=1
    )
    return out.reshape(B, N, D)

